# revision 33
# baseline (speedup 1.0000x reference)
"""Trainium2 Bass kernel for nn_Euclidean (retrieval_knn).

Computes out[b, c] = -mean_f (x[b, f] - w[c, f])^2 for x [16384, 2048] f32,
w [1000, 2048] f32, via the algebraic expansion

    out = (2/F) * (x @ w.T) - ||x_b||^2 / F - ||w_c||^2 / F

Sharding: data-parallel over the batch dim across 8 NeuronCores; w replicated.

Per-core dataflow:
  - HWDGE DMA loads fp32 tiles of x and w; VectorE casts to bf16.
  - ScalarE Square-activation with accum_out produces row norms from fp32.
  - TensorE transpose-mode matmuls (vs. a bf16 identity) put the contraction
    dim on partitions; VectorE evacuates PSUM -> SBUF.
    Walrus can encode at most one semaphore wait on the LDWEIGHTS and
    DMA-XPOSE instruction structs, so (a) the xbar DMA-transpose path is
    out (Tile's transpose-hazard serialization needs several waits), and
    (b) every transpose here is bf16 (separate LDWEIGHTS) with all its
    dependencies on the single DVE proc: DVE-cast source + DVE-evacuated
    PSUM slot. A sacrificial warmup transpose absorbs the one-time
    identity-constant wait.
  - TensorE accumulates x @ w.T in PSUM (bf16 in, fp32 accumulate); a final
    K=1 matmul against a ones-row adds -||w_c||^2/2 into PSUM.
  - ScalarE Identity-activation evacuates PSUM with scale=2/F and
    per-partition bias -||x_b||^2/F, producing the final output in one pass.
"""

import math
import os
import sys

import numpy as np

if "/opt/trn_rl_repo" not in sys.path:
    sys.path.insert(0, "/opt/trn_rl_repo")

N_CORES = 8
B_TOTAL = 16384
F = 2048
C = 1000

_cache = {}
LAST_RESULTS = None


def _legalize_waits(nc):
    """Walrus encodes at most ONE sync-wait per instruction struct, but Tile's
    sem assignment freely attaches several. Split: hoist all but the last wait
    onto standalone EventSemaphore instructions (pure sem-op carriers) placed
    immediately before the over-limit instruction on the same engine queue."""
    import bass_rust
    import concourse.mybir as mybir

    n = 0
    for f in nc.m.functions:
        for bb in f.blocks:
            newlist = []
            for inst in bb.instructions:
                si = inst.sync_info
                if si is not None and len(si.on_wait) > 1:
                    waits = list(si.on_wait)
                    for w in waits[:-1]:
                        ev = mybir.InstEventSemaphore(
                            name=f"waitsplit_{n}", ins=[], outs=[]
                        )
                        ev.engine = inst.engine
                        ev.sync_info = bass_rust.SyncInfo(on_wait=[w], on_update=[])
                        newlist.append(ev)
                        n += 1
                    inst.sync_info = bass_rust.SyncInfo(
                        on_wait=[waits[-1]], on_update=list(si.on_update)
                    )
                newlist.append(inst)
            bb.instructions = newlist
    return n


def _build():
    import concourse.bass as bass
    import concourse.mybir as mybir
    from bass_rust import add_dep_helper
    from concourse.masks import make_identity
    from concourse.tile import TileContext

    P = 128
    KT = F // P                 # 16 contraction chunks
    B = B_TOTAL // N_CORES      # 2048 batch rows per core
    BT = B // P                 # 16 batch chunks
    CP = 1024                   # padded class dim
    CT = CP // P                # 8 class chunks
    KG = 4                      # k-chunks per PSUM bank in transpose stage
    bdt = mybir.dt.bfloat16
    fdt = mybir.dt.float32
    AF = mybir.ActivationFunctionType

    nc = bass.Bass()
    x = nc.dram_tensor("x", [B, F], fdt, kind="ExternalInput")
    w = nc.dram_tensor("w", [C, F], fdt, kind="ExternalInput")
    out = nc.dram_tensor("out", [B, C], fdt, kind="ExternalOutput")

    with TileContext(nc) as tc:
        with (
            tc.tile_pool(name="consts", bufs=1) as constp,
            tc.tile_pool(name="wstage", bufs=5) as wp,
            tc.tile_pool(name="xstage", bufs=3) as xp,
            tc.tile_pool(name="evac", bufs=3) as ep,
            tc.tile_pool(name="dram", bufs=1, space="DRAM") as dp,
            tc.tile_pool(name="psum", bufs=2, space="PSUM") as pp,
        ):
            ones_row = constp.tile([1, P], bdt)
            nc.vector.memset(ones_row[:, :], 1.0)
            ident = constp.tile([P, P], bdt)
            make_identity(nc, ident[:, :])

            # Sacrificial transpose: absorbs the one-time identity-readiness
            # wait so later transposes carry only their DVE dependency.
            pwarm = pp.tile([P, P], bdt, tag="pst", bufs=4)
            nc.tensor.transpose(pwarm[:, :], ident[:, :], ident[:, :])

            wT = constp.tile([P, KT, CP], bdt)    # w^T, resident all kernel
            w2neg = constp.tile([1, CP], bdt)     # -||w_c||^2 / 2
            w2row = constp.tile([1, CP], fdt)
            w2d = dp.tile([CP, 1], fdt)

            # ---- w setup: load fp32, row norms, transpose via TensorE ----
            prev_w_c0 = None
            for j in range(CT):
                c0 = j * P
                csz = min(P, C - c0)              # 128, last chunk 104
                w_f32 = wp.tile([P, F], fdt, tag="w_f32")
                nc.sync.dma_start(out=w_f32[:csz, :], in_=w[c0 : c0 + csz, :])
                w_bf = wp.tile([P, F], bdt, tag="w_bf")
                if csz < P:
                    # pad rows feed the transpose below; keep them finite.
                    pad_base = (csz // 32) * 32
                    nc.vector.memset(w_bf[pad_base:P, :], 0.0)
                nc.vector.tensor_copy(w_bf[:csz, :], w_f32[:csz, :])
                wsq = wp.tile([P, F], bdt, tag="wsq", bufs=2)
                w2col = wp.tile([P, 1], fdt, tag="w2col")
                nc.scalar.activation(
                    wsq[:csz, :], w_f32[:csz, :], AF.Square, accum_out=w2col[:csz, :]
                )
                nc.sync.dma_start(out=w2d[c0 : c0 + csz, :], in_=w2col[:csz, :])
                # Standalone weight-loads absorb the DVE waits (bf16 cast +
                # recycled-PSUM-slot release) into the PE queue: the transpose
                # MMs below then need only their single PSUM WAW wait.
                # (LDW/MM instruction structs hold at most one wait each.)
                dums = [nc.tensor.ldweights(w_bf[:, 0:P])]
                if prev_w_c0 is not None:
                    dums.append(
                        nc.tensor.ldweights(wT[:, KT - 1, prev_w_c0 : prev_w_c0 + P])
                    )
                prev_w_c0 = c0
                for kg in range(KT // KG):
                    pstw = pp.tile([P, KG * P], bdt, tag="pst", bufs=4)
                    for q in range(KG):
                        k = kg * KG + q
                        t = nc.tensor.transpose(
                            pstw[:, q * P : (q + 1) * P],
                            w_bf[:, k * P : (k + 1) * P],
                            ident[:, :],
                        )
                        if q == 0:
                            for d in dums:
                                add_dep_helper(
                                    t.ins, d.ins, sync=False,
                                    reason="keep wait-absorber LDW before transposes",
                                )
                    # strided store into w^T
                    nc.vector.tensor_copy(
                        wT[:, kg * KG : (kg + 1) * KG, c0 : c0 + P],
                        pstw[:, :].rearrange("p (k c) -> p k c", k=KG),
                    )

            nc.sync.dma_start(
                out=w2row[0:1, 0:C], in_=w2d[0:C, :].rearrange("c one -> one c")
            )
            nc.scalar.mul(w2neg[0:1, 0:C], w2row[0:1, 0:C], -0.5)

            # one-time broadcast: w2bc[p, c] = -||w_c||^2 / F for every
            # partition, via K=1 ones-matmuls + scaled ACT evacuation.
            w2bc = constp.tile([P, CP], fdt)
            w2ps_a = pp.tile([P, 512], fdt, tag="ps_a")
            nc.tensor.matmul(
                w2ps_a[:, :], ones_row[0:1, :], w2neg[0:1, 0:512],
                start=True, stop=True,
            )
            nc.scalar.activation(
                w2bc[:, 0:512], w2ps_a[:, :], AF.Identity, scale=2.0 / F
            )
            w2ps_b = pp.tile([P, 512], fdt, tag="ps_b")
            nc.tensor.matmul(
                w2ps_b[:, 0:488], ones_row[0:1, :], w2neg[0:1, 512:1000],
                start=True, stop=True,
            )
            nc.scalar.activation(
                w2bc[:, 512:1000], w2ps_b[:, 0:488], AF.Identity, scale=2.0 / F
            )

            # ---- main loop over batch chunks ----
            inv_sqrt_f = 1.0 / math.sqrt(F)
            prev_xT = None
            for i in range(BT):
                b0 = i * P
                x_f32 = xp.tile([P, F], fdt, tag="x_f32", bufs=6)
                nc.scalar.dma_start(out=x_f32[:, :], in_=x[b0 : b0 + P, :])
                x_bf = xp.tile([P, F], bdt, tag="x_bf")
                nc.vector.tensor_copy(x_bf[:, :], x_f32[:, :])
                xsq = xp.tile([P, F], bdt, tag="xsq", bufs=2)
                x2c = xp.tile([P, 1], fdt, tag="x2c", bufs=2)
                negx2 = xp.tile([P, 1], fdt, tag="negx2", bufs=4)
                # accum_out = sum_f (x/sqrt(F))^2 = ||x_b||^2 / F
                nc.scalar.activation(
                    xsq[:, :], x_f32[:, :], AF.Square,
                    scale=inv_sqrt_f, accum_out=x2c[:, :],
                )
                nc.vector.tensor_scalar_mul(negx2[:, :], x2c[:, :], -1.0)

                xT = xp.tile([P, KT, P], bdt, tag="xT")
                dums = [nc.tensor.ldweights(x_bf[:, 0:P])]
                if prev_xT is None:
                    dums.append(
                        nc.tensor.ldweights(wT[:, KT - 1, (CT - 1) * P : CT * P])
                    )
                else:
                    dums.append(nc.tensor.ldweights(prev_xT[:, KT - 1, :]))
                prev_xT = xT
                for kg in range(KT // KG):
                    kg_dums = list(dums)
                    if i == 0 and kg > 0:
                        kg_dums.append(
                            nc.tensor.ldweights(xT[:, (kg - 1) * KG, :])
                        )
                    pst = pp.tile([P, KG * P], bdt, tag="pst", bufs=4)
                    for q in range(KG):
                        k = kg * KG + q
                        tinst = nc.tensor.transpose(
                            pst[:, q * P : (q + 1) * P],
                            x_bf[:, k * P : (k + 1) * P],
                            ident[:, :],
                        )
                        if q == 0:
                            for d in kg_dums:
                                add_dep_helper(
                                    tinst.ins, d.ins, sync=False,
                                    reason="keep wait-absorber LDW before transposes",
                                )
                    nc.vector.tensor_copy(
                        xT[:, kg * KG : (kg + 1) * KG, :], pst[:, :]
                    )

                ps_a = pp.tile([P, 512], fdt, tag="ps_a")
                ps_b = pp.tile([P, 512], fdt, tag="ps_b")
                for k in range(KT):
                    nc.tensor.matmul(
                        ps_a[:, :], xT[:, k, :], wT[:, k, 0:512],
                        start=(k == 0), stop=(k == KT - 1),
                    )
                for k in range(KT):
                    nc.tensor.matmul(
                        ps_b[:, 0:488], xT[:, k, :], wT[:, k, 512:1000],
                        start=(k == 0), stop=(k == KT - 1),
                    )

                o_sb = ep.tile([P, C], fdt, tag="o_sb")
                nc.scalar.activation(
                    o_sb[:, 0:512], ps_a[:, :], AF.Identity,
                    bias=negx2[:, 0:1], scale=2.0 / F,
                )
                nc.scalar.activation(
                    o_sb[:, 512:1000], ps_b[:, 0:488], AF.Identity,
                    bias=negx2[:, 0:1], scale=2.0 / F,
                )
                nc.vector.tensor_add(o_sb[:, 0:C], o_sb[:, 0:C], w2bc[:, 0:C])
                nc.sync.dma_start(out=out[b0 : b0 + P, :], in_=o_sb[:, :])

    return nc


def kernel(**inputs: np.ndarray) -> np.ndarray:
    global LAST_RESULTS
    x = np.ascontiguousarray(np.asarray(inputs["x"], dtype=np.float32))
    w = np.ascontiguousarray(np.asarray(inputs["w"], dtype=np.float32))
    assert x.shape == (B_TOTAL, F), x.shape
    assert w.shape == (C, F), w.shape

    from concourse.bass_utils import run_bass_kernel_spmd

    if "nc" not in _cache:
        nc = _build()
        _legalize_waits(nc)
        _cache["nc"] = nc
    nc = _cache["nc"]

    bs = B_TOTAL // N_CORES
    in_maps = [
        {"x": x[i * bs : (i + 1) * bs], "w": w} for i in range(N_CORES)
    ]
    res = run_bass_kernel_spmd(
        nc, in_maps, core_ids=list(range(N_CORES)),
        trace=bool(os.environ.get("BASS_TRACE")),
    )
    LAST_RESULTS = res
    return np.concatenate([r["out"] for r in res.results], axis=0)


if __name__ == "__main__":
    rng = np.random.default_rng(0)
    xs = rng.standard_normal((B_TOTAL, F), dtype=np.float32)
    ws = rng.standard_normal((C, F), dtype=np.float32) * math.sqrt(2.0 / F)
    o = kernel(x=xs, w=ws)
    print(o.shape, o.dtype, o[:2, :4])
